# revision 1
# baseline (speedup 1.0000x reference)
"""
CRFTagger NLL loss on 8 Trainium2 NeuronCores (Bass/Tile).

Strategy (v3: time-segmented rank-1 stitching, G=16)
----------------------------------------------------
Data-parallel over batch (16 seqs/core) + *time-parallel* within each core.
The CRF forward scan runs in the exp domain with a constant Perron shift s:

    P_{t+1} = (E^T @ P_t) * exp(feat_t),   E = exp(trans - s)  [C,C]

A serial scan step costs ~640ns of cross-engine round trip (PE SBUF access
173ns + matmul + sem + DVE PSUM access 125ns + mul + sem), so wall time is
(steps per chain) x 640ns.  Time is split into S=32 segments of G=16
steps; CRF transfer operators mix fast (segment products are numerically
rank-1; host-validated error on the final NLL ~1e-4 incl. fp8 features):

  * z-chains (31): forward recursion per segment, seeded e_start (seg 0)
    or ones (generic), advancing in TWO wide matmuls per round (staggered
    half-width groups a=[256 cols], b=[240 cols]).
  * w-chains (30): reverse recursion over the FIRST mW=5 steps of
    segments 1..30, approximating the right (q) factor of the segment
    product; host recovers boundary scales via
        P_{m_{j+1}} ~ z_j * (w_j . P_{m_j}) / (w_j . 1).
    GpSimd has no PSUM port, so a w step is MM -> Act copy (PSUM->SBUF)
    -> GpSimd mul (SBUF x SBUF), split into two half-width chains on
    alternating rounds so the relay round trip fits its cadence.
  * X-chain (1): end-ALIGNED backward recursion with history for the
    landing in the final partial segment:
        logZ_b = log(X_i . P_{m_k}) + (L+1)s,  i = L - G*k in [1,16].

Features ship as fp8e4 (halves HBM traffic; error budget validated) and
stream over THREE DMA queues (Sync + Act HWDGE, GpSimd SWDGE) to
parallelize descriptor generation (~700ns per dma_start) and per-queue
transfer bandwidth (~47GB/s each).  The w-chains reuse the z-chains'
SBUF-resident feature rows; z seeds are built on device by memset/copy;
dummy matmuls during the DMA ramp pre-warm the PE HAM clock gate.

Host: stitch boundary scales (<=30 dots/seq), land with X history, add
the gold-path score (pure gathers) -> NLL.
"""

import sys

import ml_dtypes
import numpy as np

sys.path.insert(0, "/opt/trn_rl_repo")

import concourse.bacc as bacc  # noqa: E402
import concourse.mybir as mybir  # noqa: E402
from concourse import tile  # noqa: E402
from concourse.bass_utils import run_bass_kernel_spmd  # noqa: E402
from concourse.tile_rust import add_dep_helper  # noqa: E402

B, T, C = 128, 512, 128
N_CORES = 8
BL = B // N_CORES        # 16 sequences per core
G = 16                   # steps per segment == lockstep rounds
S = T // G               # 32 segments
MW = 5                   # reverse (w) chain truncation
NZ = S - 1               # 31 z-chains (last segment covered by X landing)
NWC = S - 2              # 30 w-chains (segments 1..30)
ZC = NZ * BL             # 496 z state cols
ZA = 16 * BL             # 256: z-chains 0..15  (group a)
ZB = ZC - ZA             # 240: z-chains 16..30 (group b)
WC = NWC * BL            # 480 w state cols (chains j=1..30 -> cols 16..496)
WH = WC // 2             # 240: w half-chain width
XC = BL                  # 16 x state cols

# konst layout: E | ET | xseed(XC) | zseed0(BL)
KE, KET, KXS, KZ0 = 0, C, 2 * C, 2 * C + XC
KCOLS = 2 * C + XC + BL

_NC = None
LAST_RESULT = None

# zfeat chunk spans (rows of G): small first chunks for a fast start
_SPANS = [(0, 1), (1, 3), (3, 6), (6, 10), (10, 16)]
_XSPANS = [(0, 4), (4, 16)]
# w emission schedule: {round: [(kind, half, step), ...]}
_WSCHED = {}
for _i in range(MW):
    _WSCHED.setdefault(3 + 2 * _i, []).append(("mm", 0, _i))
    _WSCHED.setdefault(4 + 2 * _i, []).append(("relay", 0, _i))
    _WSCHED.setdefault(4 + 2 * _i, []).append(("mm", 1, _i))
    _WSCHED.setdefault(5 + 2 * _i, []).append(("relay", 1, _i))


def _build_nc():
    nc = bacc.Bacc("TRN2", target_bir_lowering=False, debug=False)
    fp32 = mybir.dt.float32
    fp8 = mybir.dt.float8e4
    bf16 = mybir.dt.bfloat16
    zfeat_h = nc.dram_tensor("zfeat", [C, G, ZC], fp8, kind="ExternalInput")
    xfeat_h = nc.dram_tensor("xfeat", [C, G * BL], fp8, kind="ExternalInput")
    konst_h = nc.dram_tensor("konst", [C, KCOLS], bf16, kind="ExternalInput")
    zout_h = nc.dram_tensor("zout", [C, ZC], bf16, kind="ExternalOutput")
    wout_h = nc.dram_tensor("wout", [C, WC], bf16, kind="ExternalOutput")
    xh_h = nc.dram_tensor("xh", [C, G * BL], bf16, kind="ExternalOutput")

    with tile.TileContext(nc) as tc:
        with (
            tc.tile_pool(name="consts", bufs=1) as consts,
            tc.tile_pool(name="zf", bufs=len(_SPANS)) as zfp,
            tc.tile_pool(name="xf", bufs=len(_XSPANS)) as xfp,
            tc.tile_pool(name="state", bufs=1) as state,
            tc.tile_pool(name="psA", bufs=2, space="PSUM") as psA,
            tc.tile_pool(name="psB", bufs=2, space="PSUM") as psB,
            tc.tile_pool(name="psW", bufs=2, space="PSUM") as psW,
        ):
            konst = consts.tile([C, KCOLS], bf16)
            nc.sync.dma_start(out=konst[:], in_=konst_h[:])
            emat = konst[:, KE:KE + C]
            ematT = konst[:, KET:KET + C]

            ones = consts.tile([C, WC], bf16)
            nc.gpsimd.memset(ones[:], 1.0)   # first: warm MMs depend on it
            seedA = consts.tile([C, ZA], bf16)

            zf_tiles = [None] * len(_SPANS)
            xf_tiles = [None] * len(_XSPANS)

            def load_z(i, eng):
                lo, hi = _SPANS[i]
                f = zfp.tile([C, (hi - lo) * ZC], fp8)
                eng.dma_start(
                    out=f[:],
                    in_=zfeat_h[:, lo:hi, :].rearrange("c t b -> c (t b)"),
                )
                zf_tiles[i] = f

            def load_x(i, eng):
                lo, hi = _XSPANS[i]
                f = xfp.tile([C, (hi - lo) * BL], fp8)
                eng.dma_start(out=f[:], in_=xfeat_h[:, lo * BL:hi * BL])
                xf_tiles[i] = f

            # three DMA queues; ~47GB/s each, so balance bytes and put
            # first-needed chunks first per queue
            load_z(0, nc.sync)
            load_x(0, nc.gpsimd)
            load_z(1, nc.scalar)
            load_z(2, nc.sync)
            load_z(3, nc.gpsimd)
            load_z(4, nc.scalar)
            load_x(1, nc.gpsimd)
            nc.scalar.copy(seedA[:, 0:BL], konst[:, KZ0:KZ0 + BL])
            nc.scalar.copy(seedA[:, BL:ZA], ones[:, 0:ZA - BL])

            # warm the PE (HAM clock gate) during the DMA ramp: ~10 dummy
            # matmuls keep it busy >3.4us so real matmuls run at 2.4GHz
            for _ in range(5):
                wm = psW.tile([C, WH], fp32)
                nc.tensor.matmul(wm[:], ones[:, 0:C], ones[:, 0:WC:2],
                                 start=True, stop=True)

            def zrow(r, c0, c1):
                for (lo, hi), f in zip(_SPANS, zf_tiles):
                    if lo <= r < hi:
                        return f[:, (r - lo) * ZC + c0:(r - lo) * ZC + c1]
                raise AssertionError(r)

            def xslice(k):
                for (lo, hi), f in zip(_XSPANS, xf_tiles):
                    if lo <= k < hi:
                        return f[:, (k - lo) * BL:(k - lo + 1) * BL]
                raise AssertionError(k)

            # states: 2 explicit slots each (double buffer)
            za = state.tile([C, 2 * ZA], bf16)
            zb = state.tile([C, 2 * ZB], bf16)
            ws = state.tile([C, 2 * WC], bf16)
            wtmp = state.tile([C, 2 * WC], bf16)
            xh = state.tile([C, G * BL], bf16)   # X history IS the x state

            def slot(tile_, k, w):
                i = (k % 2) * w
                return tile_[:, i:i + w]

            def za_step(k):
                m = psA.tile([C, ZA], fp32)
                rhs = seedA[:] if k == 0 else slot(za, k, ZA)
                nc.tensor.matmul(m[:], emat, rhs, start=True, stop=True)
                return nc.vector.tensor_mul(
                    slot(za, k + 1, ZA), zrow(k, 0, ZA), m[:]
                )

            def zb_step(k, pin, mbx):
                m = mbx[:, 0:ZB]
                rhs = (ones[:, 0:ZB] if k == 0
                       else slot(zb, k, ZB))
                mm = nc.tensor.matmul(m, emat, rhs, start=True, stop=True)
                if pin is not None:
                    add_dep_helper(mm.ins, pin.ins, sync=True,
                                   reason="phase pin")
                return nc.vector.tensor_mul(
                    slot(zb, k + 1, ZB), zrow(k, ZA, ZC), m
                )

            def x_step(k, mbx):
                m = mbx[:, ZB:ZB + XC]
                rhs = (konst[:, KXS:KXS + XC] if k == 0
                       else xh[:, (k - 1) * BL:k * BL])
                nc.tensor.matmul(m, ematT, rhs, start=True, stop=True)
                return nc.vector.tensor_mul(
                    xh[:, k * BL:(k + 1) * BL], xslice(k), m
                )

            w_ps = [[None] * MW, [None] * MW]

            def wslot(h, k):
                i = (k % 2) * WC + h * WH
                return ws[:, i:i + WH]

            def w_mm(h, i, pin):
                m = psW.tile([C, WH], fp32)
                c0 = BL + h * WH
                rhs = (zrow(MW - 1, c0, c0 + WH) if i == 0
                       else wslot(h, i))
                mm = nc.tensor.matmul(m[:], ematT, rhs, start=True, stop=True)
                if pin is not None:
                    add_dep_helper(mm.ins, pin.ins, sync=True,
                                   reason="w phase pin")
                w_ps[h][i] = m

            def w_relay(h, i):
                c0 = BL + h * WH
                tmp = wtmp[:, h * WH:(h + 1) * WH]
                nc.scalar.copy(tmp, w_ps[h][i][:])
                fop = (zrow(MW - 2 - i, c0, c0 + WH) if i < MW - 1
                       else ones[:, 0:WH])
                nc.gpsimd.tensor_mul(wslot(h, i + 1), fop, tmp)

            # lockstep rounds; zb/x phase lags za by one round with an
            # explicit cross-phase pin so the phases interleave on PE/DVE;
            # w advances off-path per _WSCHED (long Act+GpSimd relay)
            prev_a = None
            for k in range(G + 2):
                if k < G:
                    ta = za_step(k)
                if 1 <= k <= G:
                    mbx = psB.tile([C, ZB + XC], fp32)
                    x_step(k - 1, mbx)
                    zb_step(k - 1, prev_a, mbx)
                for kind, h, i in _WSCHED.get(k, ()):
                    if kind == "mm":
                        w_mm(h, i, prev_a)
                    else:
                        w_relay(h, i)
                if k == 14:  # early partial ship of X history (Act queue)
                    nc.scalar.dma_start(out=xh_h[:, 0:12 * BL],
                                        in_=xh[:, 0:12 * BL])
                if k < G:
                    prev_a = ta

            nc.scalar.dma_start(out=xh_h[:, 12 * BL:], in_=xh[:, 12 * BL:])
            nc.sync.dma_start(out=zout_h[:, 0:ZA], in_=slot(za, G, ZA))
            nc.scalar.dma_start(out=zout_h[:, ZA:ZC], in_=slot(zb, G, ZB))
            nc.sync.dma_start(out=wout_h[:, 0:WH], in_=wslot(0, MW))
            nc.scalar.dma_start(out=wout_h[:, WH:WC], in_=wslot(1, MW))
    nc.compile()
    return nc


def _get_nc():
    global _NC
    if _NC is None:
        _NC = _build_nc()
    return _NC


def _shift_constant(transitions: np.ndarray) -> float:
    tm = transitions.astype(np.float64)
    mx = tm.max()
    Et = np.exp(tm - mx)
    v = np.ones(C) / C
    r = 1.0
    for _ in range(200):
        w = Et.T @ v
        r = np.linalg.norm(w)
        v = w / r
    return float(np.log(r) + mx + 0.5)


def kernel(feats, mask, tags, transitions):
    global LAST_RESULT
    feats = np.asarray(feats, dtype=np.float32)
    mask = np.asarray(mask, dtype=np.int32)
    tags = np.asarray(tags, dtype=np.int32)
    transitions = np.asarray(transitions, dtype=np.float32)

    s = _shift_constant(transitions)
    with np.errstate(under="ignore"):
        E64 = np.exp(transitions.astype(np.float64) - s)
        emat = E64.astype(np.float32).astype(ml_dtypes.bfloat16)
        vstop = E64[:, C - 1].astype(np.float32)
        fe = np.exp(feats)                       # [B,T,C] float32

    lengths = mask.sum(1)                        # [B]

    konst = np.zeros((C, KCOLS), dtype=ml_dtypes.bfloat16)
    konst[:, KE:KE + C] = emat
    konst[:, KET:KET + C] = emat.T
    konst[C - 2, KZ0:KZ0 + BL] = 1.0             # segment-0 seed: START

    # z feature rows: [C, G, NZ*BL] per core; col j*BL+b = fe[b, j*G+k, :]
    fe_r = fe.reshape(B, S, G, C)                # [B, S, G, C]
    # x stream (end-aligned): slice k col b = fe[b, L-2-k] (ones pad)
    kk = np.arange(G)[None, :]                   # [1,G]
    tidx = lengths[:, None] - 2 - kk             # [B,G]
    valid = tidx >= 0
    xstr = np.ones((B, G, C), dtype=np.float32)
    xstr[valid] = fe[np.nonzero(valid)[0], tidx[valid]]
    xseed = fe[np.arange(B), lengths - 1] * vstop[None, :]   # [B,C]

    in_maps = []
    for c in range(N_CORES):
        b0 = c * BL
        zf = fe_r[b0:b0 + BL, :NZ].transpose(3, 2, 1, 0).reshape(C, G, ZC)
        xf = xstr[b0:b0 + BL].transpose(2, 1, 0).reshape(C, G * BL)
        kc = konst.copy()
        kc[:, KXS:KXS + XC] = xseed[b0:b0 + BL].T.astype(ml_dtypes.bfloat16)
        in_maps.append({
            "zfeat": np.ascontiguousarray(zf.astype(ml_dtypes.float8_e4m3)),
            "xfeat": np.ascontiguousarray(xf.astype(ml_dtypes.float8_e4m3)),
            "konst": kc,
        })

    nc = _get_nc()
    res = run_bass_kernel_spmd(nc, in_maps, core_ids=list(range(N_CORES)))
    LAST_RESULT = res

    # ---- host stitch ----
    logZ = np.zeros(B, dtype=np.float64)
    for c in range(N_CORES):
        zf = np.asarray(res.results[c]["zout"]).astype(np.float64)  # [C,ZC]
        wf = np.asarray(res.results[c]["wout"]).astype(np.float64)  # [C,WC]
        xhv = np.asarray(res.results[c]["xh"]).astype(np.float64)   # [C,G*BL]
        for b in range(BL):
            bg = c * BL + b
            L = int(lengths[bg])
            kseg = (L - 1) // G
            i = L - G * kseg                     # 1..G
            xs = xhv[:, (i - 1) * BL + b].copy()
            t = L - 1 - i
            if t >= 0:
                xs /= fe[bg, t]
            if kseg == 0:
                num = xs[C - 2]
                lsc = 0.0
            else:
                dirP = zf[:, (kseg - 1) * BL + b]
                num = float(xs @ dirP)
                lsc = 0.0
                for j in range(1, kseg):
                    wj = wf[:, (j - 1) * BL + b]
                    lsc += np.log(wj @ zf[:, (j - 1) * BL + b]) - np.log(wj.sum())
            logZ[bg] = np.log(num) + lsc + (L + 1) * s
    fwd = np.float32(logZ.astype(np.float32).sum())

    # ---- gold-path score (host; pure gather/sum) ----
    r = np.arange(B)
    pad_start = np.concatenate([np.full((B, 1), C - 2, tags.dtype), tags], axis=1)
    pad_stop = np.concatenate([tags, np.full((B, 1), C - 1, tags.dtype)], axis=1)
    pad_stop[r, lengths] = C - 1
    tvals = transitions[pad_start, pad_stop]
    t_score = np.cumsum(tvals, axis=1)[r, lengths].sum(dtype=np.float32)
    fg = np.take_along_axis(feats, tags[:, :, None], axis=2)[..., 0]
    f_score = np.where(mask.astype(bool), fg, np.float32(0.0)).sum(dtype=np.float32)

    nll = (np.float32(fwd) - (t_score + f_score)) / np.float32(B)
    return np.array(nll, dtype=np.float32)



# revision 7
# speedup vs baseline: 1.6310x; 1.6310x over previous
"""
CRFTagger NLL loss on 8 Trainium2 NeuronCores (Bass/Tile).

Strategy (v4: live-packed z-chains, G=4, host stitch)
-----------------------------------------------------
The CRF forward scan runs in the exp domain with a constant Perron shift s:

    P_{t+1} = (E^T @ P_t) * exp(feat_t),   E = exp(trans - s)  [C,C]

Time is split into S = T/G segments of G=4 steps. Per sequence b only
segments j < kseg_b = (L_b-1)//G carry information (random lengths =>
~half the (b, j) chains of a dense layout are dead), so the device runs
ONE slot per LIVE chain, packed and load-balanced across the 8 cores
(~1010 slots/core vs 496*2 col-equivalents in the dense G=16 layout).

Device work per round k (G rounds): two staggered half-width groups so
PE matmul and DVE multiply pipeline:

    psum = E^T @ state      (PE,  bf16, ~0.55 ns/col)
    state = feat_k * psum   (DVE, fp8 x psum-fp32 -> bf16, ~1.04 ns/col)

Everything else is host-side (it depends only on features + E, never on
device state): the end-aligned landing vectors x_b (<= G exact backward
steps), the rank-1 stitch test vectors w_{b,j} (MW=1 reverse step), the
boundary-scale stitching, and the gold-path score. Segment transfer
operators are numerically rank-1 after even 4 steps (host-validated:
stitched NLL rel err ~2e-4, dominated by the fp8 feature quantization,
vs a 2e-2 gate):

    logZ_b = log(x_b . z_{k-1}) + sum_{j=1}^{k-1} [log(w_j . z_{j-1})
             - log(w_j . 1)] + (L_b + 1) s

Features ship as fp8e4 over four DMA queues (sync/scalar/vector/gpsimd,
one G-chunk each) to parallelize descriptor generation; E ships bf16 on
the fastest (sync) queue; seeds (e_start for j=0 slots, ones elsewhere)
are built on device by memsets; dummy matmuls pre-warm the PE clock
during the DMA ramp.
"""

import sys

import ml_dtypes
import numpy as np

sys.path.insert(0, "/opt/trn_rl_repo")

import concourse.bacc as bacc  # noqa: E402
import concourse.mybir as mybir  # noqa: E402
from concourse import tile  # noqa: E402
from concourse.bass_utils import run_bass_kernel_spmd  # noqa: E402

B, T, C = 128, 512, 128
N_CORES = 8
BL = B // N_CORES        # 16 sequences per core
MW = 1                   # host-side reverse test-vector steps

_NC = {}
LAST_RESULT = None


def _build_nc(G, W, Wa):
    """Bass program: G rounds over two staggered groups of live z-slots.

    Group a = slots [0, Wa) (first BL slots are the j=0 chains, seeded
    e_start), group b = slots [Wa, W) (seeded ones).
    """
    Wb = W - Wa
    nc = bacc.Bacc("TRN2", target_bir_lowering=False, debug=False)
    fp32 = mybir.dt.float32
    fp8 = mybir.dt.float8e4
    bf16 = mybir.dt.bfloat16
    zfeat_h = nc.dram_tensor("zfeat", [C, G, W], fp8, kind="ExternalInput")
    konst_h = nc.dram_tensor("konst", [C, C + BL], bf16, kind="ExternalInput")
    zout_h = nc.dram_tensor("zout", [C, W], bf16, kind="ExternalOutput")

    with tile.TileContext(nc) as tc:
        with (
            tc.tile_pool(name="consts", bufs=1) as consts,
            tc.tile_pool(name="zf", bufs=G) as zfp,
            tc.tile_pool(name="state", bufs=1) as state,
            tc.tile_pool(name="psA", bufs=2, space="PSUM") as psA,
            tc.tile_pool(name="psB", bufs=2, space="PSUM") as psB,
            tc.tile_pool(name="psW", bufs=2, space="PSUM") as psW,
        ):
            emat = consts.tile([C, C], bf16)
            seedA = consts.tile([C, Wa], bf16)
            nc.sync.dma_start(out=emat[:], in_=konst_h[:, 0:C])
            nc.sync.dma_start(out=seedA[:, 0:BL], in_=konst_h[:, C:C + BL])

            ones = consts.tile([C, Wb], bf16)
            nc.gpsimd.memset(ones[:], 1.0)   # first: warm MMs depend on it
            nc.gpsimd.memset(seedA[:, BL:Wa], 1.0)

            # four DMA queues; first-needed chunks on the fastest queues
            zf_tiles = [None] * G
            engs = {0: nc.scalar, 1: nc.gpsimd, 2: nc.sync, 3: nc.scalar}

            def load_z(i, eng):
                f = zfp.tile([C, W], fp8)
                eng.dma_start(out=f[:], in_=zfeat_h[:, i, :])
                zf_tiles[i] = f

            for i in range(G):
                load_z(i, engs[i % 4])

            # warm the PE (HAM clock gate) during the DMA ramp
            wsw = min(Wb, 256)
            for _ in range(4):
                wm = psW.tile([C, wsw], fp32)
                nc.tensor.matmul(wm[:], ones[:, 0:min(Wb, C)],
                                 ones[:, 0:wsw], start=True, stop=True)

            # states: 2 explicit slots each (double buffer)
            za = state.tile([C, 2 * Wa], bf16)
            zb = state.tile([C, 2 * Wb], bf16)

            def slot(tile_, k, w):
                i = (k % 2) * w
                return tile_[:, i:i + w]

            def za_step(k):
                m = psA.tile([C, Wa], fp32)
                rhs = seedA[:] if k == 0 else slot(za, k, Wa)
                nc.tensor.matmul(m[:], emat[:], rhs, start=True, stop=True)
                nc.vector.tensor_mul(
                    slot(za, k + 1, Wa), zf_tiles[k][:, 0:Wa], m[:]
                )

            def zb_step(k):
                m = psB.tile([C, Wb], fp32)
                rhs = ones[:] if k == 0 else slot(zb, k, Wb)
                nc.tensor.matmul(m[:], emat[:], rhs, start=True, stop=True)
                nc.vector.tensor_mul(
                    slot(zb, k + 1, Wb), zf_tiles[k][:, Wa:W], m[:]
                )

            # staggered rounds: group b lags group a by one round so PE/DVE
            # of the two groups interleave
            for k in range(G + 1):
                if k < G:
                    za_step(k)
                if k >= 1:
                    zb_step(k - 1)

            nc.sync.dma_start(out=zout_h[:, 0:Wa], in_=slot(za, G, Wa))
            nc.scalar.dma_start(out=zout_h[:, Wa:W], in_=slot(zb, G, Wb))
    nc.compile()
    return nc


def _get_nc(G, W, Wa):
    key = (G, W, Wa)
    if key not in _NC:
        _NC[key] = _build_nc(G, W, Wa)
    return _NC[key]


def _shift_constant(transitions: np.ndarray) -> float:
    tm = transitions.astype(np.float64)
    mx = tm.max()
    Et = np.exp(tm - mx)
    v = np.ones(C) / C
    r = 1.0
    for _ in range(200):
        w = Et.T @ v
        r = np.linalg.norm(w)
        v = w / r
    return float(np.log(r) + mx + 0.5)


def _pack(lengths, G):
    """Assign sequences to cores (16 each, balanced by live-chain count)
    and live chains to slots.

    Returns (W, Wa, core_seqs[8][16], slotmap) where slotmap[(b, j)] =
    (core, slot). Slots [0, BL) of each core are its j=0 chains in local
    seq order (dummy for kseg=0 seqs).
    """
    kseg = (lengths - 1) // G
    order = np.argsort(-kseg, kind="stable")
    loads = [0] * N_CORES
    core_seqs = [[] for _ in range(N_CORES)]
    for b in order:
        cands = [c for c in range(N_CORES) if len(core_seqs[c]) < BL]
        c = min(cands, key=lambda c: loads[c])
        core_seqs[c].append(int(b))
        loads[c] += max(int(kseg[b]) - 1, 0)
    W = BL + max(loads)
    Wa = (W + 1) // 2
    slotmap = {}
    for c in range(N_CORES):
        for i, b in enumerate(core_seqs[c]):
            if kseg[b] >= 1:
                slotmap[(b, 0)] = (c, i)
        pos = BL
        for b in core_seqs[c]:
            for j in range(1, int(kseg[b])):
                slotmap[(b, j)] = (c, pos)
                pos += 1
    return W, Wa, core_seqs, slotmap


def kernel(feats, mask, tags, transitions):
    global LAST_RESULT
    feats = np.asarray(feats, dtype=np.float32)
    mask = np.asarray(mask, dtype=np.int32)
    tags = np.asarray(tags, dtype=np.int32)
    transitions = np.asarray(transitions, dtype=np.float32)

    s = _shift_constant(transitions)
    E64 = np.exp(transitions.astype(np.float64) - s)
    emat = E64.astype(np.float32).astype(ml_dtypes.bfloat16)
    E_f = np.asarray(emat, dtype=np.float64)        # device-visible E
    vstop = E64[:, C - 1]
    with np.errstate(under="ignore"):
        fe = np.exp(feats.astype(np.float64))        # [B,T,C] float64
        fe8 = fe.astype(np.float32).astype(ml_dtypes.float8_e4m3)

    lengths = mask.sum(1)                            # [B]
    r = np.arange(B)

    # pick the smallest G whose packed width fits two PSUM banks
    for G in (4, 8, 16, 32):
        W, Wa, core_seqs, slotmap = _pack(lengths, G)
        if Wa <= 512:
            break
    kseg = (lengths - 1) // G

    # z feature stream: zfeat[:, k, slot] = fe8[b_slot, j_slot*G + k, :]
    in_maps = []
    for c in range(N_CORES):
        zf = np.ones((W, G, C), dtype=ml_dtypes.float8_e4m3)
        for (b, j), (cc, slotc) in slotmap.items():
            if cc == c:
                zf[slotc] = fe8[b, j * G:(j + 1) * G]
        konst = np.zeros((C, C + BL), dtype=ml_dtypes.bfloat16)
        konst[:, 0:C] = emat
        konst[C - 2, C:C + BL] = 1.0          # START one-hot seed block
        in_maps.append({
            "zfeat": np.ascontiguousarray(zf.transpose(2, 1, 0)),
            "konst": konst,
        })

    nc = _get_nc(G, W, Wa)
    res = run_bass_kernel_spmd(nc, in_maps, core_ids=list(range(N_CORES)))
    LAST_RESULT = res

    # all shipped states as one [N_CORES*W, C] float64 matrix
    Zall = np.concatenate(
        [np.asarray(res.results[c]["zout"]).astype(np.float64).T
         for c in range(N_CORES)], axis=0)           # [8*W, C]

    def gslot(b, j):
        c, sl = slotmap[(b, j)]
        return c * W + sl

    # ---- host x landing vectors (exact backward walk, <= G steps) ----
    # x_0 = fe[b, L-1] * E[:, stop]; x_{k+1} = (E @ x_k) * fe[b, L-2-k]
    # landing index i = L - G*kseg in [1, G]; Xs = x_{i-1}
    i_land = lengths - G * kseg
    X = fe[r, lengths - 1] * vstop[None, :]
    Xs = np.zeros((B, C))
    Xs[i_land == 1] = X[i_land == 1]
    for step in range(1, G):
        tpos = lengths - 1 - step
        valid = tpos >= 0
        f = np.ones((B, C))
        f[valid] = fe[np.nonzero(valid)[0], tpos[valid]]
        X = (X @ E64.T) * f
        done = i_land == step + 1
        Xs[done] = X[done]

    # ---- host w test vectors (MW reverse steps from ones) + stitch ----
    wb = np.concatenate([np.full(max(int(kseg[b]) - 1, 0), b) for b in r])
    wj = np.concatenate([np.arange(1, int(kseg[b])) for b in r]) \
        if len(wb) else np.zeros(0, int)
    Wv = np.ones((len(wb), C))
    for t in range(MW - 1, -1, -1):
        Wv = (Wv * fe[wb, wj * G + t]) @ E64.T

    logZ = np.zeros(B)
    has = kseg >= 1
    bidx = np.nonzero(has)[0]
    zlast = Zall[[gslot(b, int(kseg[b]) - 1) for b in bidx]]
    num = (Xs[bidx] * zlast).sum(1)
    logZ[bidx] = np.log(num) + (lengths[bidx] + 1) * s
    logZ[~has] = np.log(Xs[~has, C - 2]) + (lengths[~has] + 1) * s
    if len(wb):
        zprev = Zall[[gslot(b, j - 1) for b, j in zip(wb, wj)]]
        terms = np.log((Wv * zprev).sum(1)) - np.log(Wv.sum(1))
        np.add.at(logZ, wb, terms)
    fwd = np.float32(logZ.astype(np.float32).sum())

    # ---- gold-path score (host; pure gather/sum) ----
    pad_start = np.concatenate([np.full((B, 1), C - 2, tags.dtype), tags], axis=1)
    pad_stop = np.concatenate([tags, np.full((B, 1), C - 1, tags.dtype)], axis=1)
    pad_stop[r, lengths] = C - 1
    tvals = transitions[pad_start, pad_stop]
    t_score = np.cumsum(tvals, axis=1)[r, lengths].sum(dtype=np.float32)
    fg = np.take_along_axis(feats, tags[:, :, None], axis=2)[..., 0]
    f_score = np.where(mask.astype(bool), fg, np.float32(0.0)).sum(dtype=np.float32)

    nll = (np.float32(fwd) - (t_score + f_score)) / np.float32(B)
    return np.array(nll, dtype=np.float32)
